# revision 36
# baseline (speedup 1.0000x reference)
"""BranchingAttention (ViewFormer) Trainium2 Bass kernel.

Problem: two token streams x0 (trunk) / x1, fused qkv projection
(w_attn packs v|q|k), block-causal multi-end attention:
  query token t in block i of branch e attends ALL tokens of trunk
  blocks j < i plus causally (u <= t) its own branch's block i,
joint softmax, out projection.  Returns (out0, out1).

Sharding (8 cores): data-parallel over batch (B=2) x tensor-parallel
over 4 head-groups of 3 heads.  Each core computes BOTH branches for
its 3 heads and emits partial projections; the host sums the 4 head
-group partials per (branch, batch) and adds b_proj.

Per-core device kernel (uniform SPMD program, bf16 matmuls, fp32 psum):
  - qkv: psum[grp] = sum_dchunk Wg[dc].T @ xT[dc]; ACT adds bias and
    copies psum -> QV sbuf (bf16).
    groups g0=[q0|k2] g1=[q1|v0] g2=[q2|v1] g3=[k0|v2] g4=[k1|-]
  - v natural layout via PE transposes + ones column -> AV lhsT [128,65]
  - attention is tiled into 18 units (branch, head, query-chunk of
    <=512).  Each unit: scoresT into [128,1024] psum regions, ACT exp
    -> bf16 expT, DVE causal-mask mults, AV accumulate into a
    one-bank [128,512] O (denominator rides as row 64).
  - software pipeline: unit u's scores are emitted before unit u-1's
    AV, so by the time the in-order PE reaches an AV matmul its
    exp/mask inputs are complete; O psum is 4-deep so normalization
    (DVE fast reciprocal -> DRAM broadcast -> DVE mult, delayed two
    units) never stalls the PE.
  - proj (at the end): out[tok,768] = aT(2 k-chunks: 128+64).T @ Wp,
    psum -> ACT copy -> sbuf fp32 -> DRAM partials.
"""
import sys

sys.path.insert(0, "/opt/trn_rl_repo")

import ml_dtypes
import numpy as np

import concourse.bacc as bacc
import concourse.mybir as mybir
import concourse.tile as tile
from concourse.bass_utils import run_bass_kernel_spmd

F32 = mybir.dt.float32
BF16 = mybir.dt.bfloat16
NBF = ml_dtypes.bfloat16

B, NB, BS = 2, 20, 64
D, H = 768, 12
DH = D // H                      # 64
T = NB * BS                      # 1280 tokens per (batch, branch)
DC = D // 128                    # 6 d-chunks
NG = 5                           # qkv col groups of 128
NP = NB // 2                     # 10 key-chunk pairs
H3 = 3                           # heads per core
REG_W = 1024                     # scoresT psum region width (2 banks)
QCHUNKS = ((0, 512), (512, 512), (1024, 256))
SCALE = 1.0 / np.sqrt(DH)
COPY = mybir.ActivationFunctionType.Copy
IDENT = mybir.ActivationFunctionType.Identity


# ---------------------------------------------------------------- device IR


def _unit_plan(e, qlo, qw):
    """Ordered (kind, idx, qoff, width) score segments for one attention
    unit (branch e, queries [qlo, qlo+qw)).

    kind: 's' self (keys = k_e pair idx), 't' trunk (keys = k0 pair idx).
    The first segment is trunk chunk 0, which covers the whole query
    span (except e=1 cols [0,64), handled by a zero-fill) so its AV
    matmul can open the O psum bank with start=True.
    """
    qhi = qlo + qw
    segs = []
    if e == 0:
        for c in range(NP):
            lo = max(128 * c, qlo)
            if lo < qhi:
                segs.append(("t", c, lo, qhi - lo))
    else:
        lo0 = max(64, qlo)
        segs.append(("t", 0, lo0, qhi - lo0))
        for p in range(NP):
            lo = max(128 * p, qlo)
            hi = min(128 * p + 128, qhi)
            if lo < hi:
                segs.append(("s", p, lo, hi - lo))
        for c in range(1, NP):
            lo = max(128 * c + 64, qlo)
            if lo < qhi:
                segs.append(("t", c, lo, qhi - lo))
    return segs


def _pack_regions(segs):
    """Pack segment pieces into [128, REG_W] psum regions.

    Returns list of (parts, used); part = (kind, idx, qoff, loc, w).
    """
    regions = []
    cur, used = [], 0
    for kind, idx, qoff, width in segs:
        off = 0
        while off < width:
            if REG_W - used < 128:
                regions.append((cur, used))
                cur, used = [], 0
            w = min(width - off, REG_W - used)
            cur.append((kind, idx, qoff + off, used, w))
            used += w
            off += w
    if cur:
        regions.append((cur, used))
    return regions


def build():
    nc = bacc.Bacc()

    xt = nc.dram_tensor("xt", [DC, 128, T], BF16, kind="ExternalInput")
    xs = nc.dram_tensor("xs", [DC, 128, T], BF16, kind="ExternalInput")
    wq = nc.dram_tensor("wq", [DC, 128, NG * 128], BF16, kind="ExternalInput")
    bq = nc.dram_tensor("bq", [128, NG], F32, kind="ExternalInput")
    wp = nc.dram_tensor("wp", [2, 128, D], BF16, kind="ExternalInput")
    # cb packs every small bf16 constant into one DMA:
    # cols 0:64 i2(base64 identity), 64:192 m0, 192:256 m1, 256:384 ms,
    # 384:414 vo(ones), row 0 cols 414:926 ones(on), 926:991 zeros(zv)
    cb = nc.dram_tensor("cb", [128, 991], BF16, kind="ExternalInput")
    o0 = nc.dram_tensor("o0", [T, D], F32, kind="ExternalOutput")
    o1 = nc.dram_tensor("o1", [T, D], F32, kind="ExternalOutput")
    outs = (o0, o1)

    with tile.TileContext(nc) as tc:
        with (
            tc.tile_pool(name="consts", bufs=1) as cp,
            tc.tile_pool(name="big", bufs=1) as bp,
            tc.tile_pool(name="xtp", bufs=2 * DC) as xtp,
            tc.tile_pool(name="expp", bufs=8) as expp,
            tc.tile_pool(name="outst", bufs=3) as outst,
            tc.tile_pool(name="dnp", bufs=4) as dnp,
            tc.tile_pool(name="rrecp", bufs=4) as rrecp,
            tc.tile_pool(name="rbp", bufs=4) as rbp,
            tc.tile_pool(name="scrp", bufs=4, space="DRAM") as scrp,
            tc.tile_pool(name="work", bufs=2, space="PSUM") as work,
            tc.tile_pool(name="psO", bufs=4, space="PSUM") as psO,
        ):
            # ---- first-needed DMAs first: the Sync engine issues DMA
            # descriptors serially (~600ns each), so emission order is the
            # startup critical path
            wq_sb = cp.tile([128, DC, NG * 128], BF16)
            xtiles = {}

            def load_x(s, xdram, dc):
                xtile = xtp.tile([128, T], BF16, tag="xt")
                nc.sync.dma_start(xtile[:], xdram[dc])
                xtiles[(s, dc)] = xtile

            nc.sync.dma_start(wq_sb[:, 0, :], wq[0])
            load_x(0, xt, 0)
            for dc in range(1, DC):
                nc.sync.dma_start(wq_sb[:, dc, :], wq[dc])
                load_x(0, xt, dc)
            bq_sb = cp.tile([128, NG], F32)
            nc.sync.dma_start(bq_sb[:], bq[:])
            cb_sb = cp.tile([128, 991], BF16)
            nc.sync.dma_start(cb_sb[:], cb[:])
            i2_sb = cb_sb[:, 0:64]
            m0_sb = cb_sb[:, 64:192]
            m1_sb = cb_sb[:, 192:256]
            ms_sb = cb_sb[:, 256:384]
            on_sb = cb_sb[0:1, 414:926]
            zv_sb = cb_sb[0:1, 926:991]
            wp_sb = cp.tile([128, 2, D], BF16)
            for c in range(2):
                nc.sync.dma_start(wp_sb[:, c, :], wp[c])
            for dc in range(DC):
                load_x(1, xs, dc)

            # ---- persistent per-source tensors
            QV, K2, VNA = [], [], []
            for s in range(2):
                qv = bp.tile([128, NG, T], BF16, name=f"qv{s}")
                k2 = bp.tile([64, T], BF16, name=f"k2{s}")
                vna = bp.tile([128, NP, H3, 65], BF16, name=f"vna{s}")
                nc.vector.tensor_copy(
                    vna[:, :, :, 64:65],
                    cb_sb[:, 384:414].rearrange(
                        "p (a c d) -> p a c d", c=H3, d=1
                    ),
                )
                QV.append(qv)
                K2.append(k2)
                VNA.append(vna)
            aT01 = [bp.tile([128, T], BF16, name=f"a01_{e}") for e in range(2)]
            aT2 = [bp.tile([64, T], BF16, name=f"a2_{e}") for e in range(2)]

            # views --------------------------------------------------------
            def qT(s, h):
                return QV[s][0:64, h, :]

            def kT(s, h):
                return (QV[s][0:64, 3, :], QV[s][0:64, 4, :], K2[s][:, :])[h]

            def vT(s, h):  # partition base 64
                return QV[s][64:128, 1 + h, :]

            # ---- phase 1: qkv projections -> QV (source 0 up front;
            # source 1 is interleaved into branch-0 attention as PE
            # filler so the HAM activity monitor never throttles)
            def qkv_region(s, g, lo, w, on_act):
                pg = work.tile([128, REG_W], F32, tag="work")
                for dc in range(DC):
                    for p0 in range(0, w, 512):
                        pw = min(512, w - p0)
                        nc.tensor.matmul(
                            pg[:, p0 : p0 + pw],
                            wq_sb[:, dc, 128 * g : 128 * (g + 1)],
                            xtiles[(s, dc)][:, lo + p0 : lo + p0 + pw],
                            start=(dc == 0),
                            stop=(dc == DC - 1),
                        )
                if on_act:
                    nc.scalar.activation(
                        QV[s][:, g, lo : lo + w],
                        pg[:, 0:w],
                        IDENT,
                        bias=bq_sb[:, g : g + 1],
                        scale=1.0,
                    )
                else:
                    nc.vector.tensor_scalar_add(
                        QV[s][:, g, lo : lo + w],
                        pg[:, 0:w],
                        bq_sb[:, g : g + 1],
                    )

            def k2_realign(s):
                nc.sync.dma_start(K2[s][:], QV[s][64:128, 0, :])

            def v_transpose(s, h):
                pt = work.tile([128, REG_W], BF16, tag="work")
                for tch in range(NP):
                    nc.tensor.transpose(
                        pt[:, 64 * tch : 64 * tch + 64],
                        vT(s, h)[:, 128 * tch : 128 * (tch + 1)],
                        i2_sb[64:128, :],
                    )
                nc.vector.tensor_copy(
                    VNA[s][:, :, h, 0:64],
                    pt[:, 0:640].rearrange("p (tc d) -> p tc d", d=64),
                )

            for g in range(NG):
                for li, (lo, w) in enumerate(((0, 1024), (1024, 256))):
                    qkv_region(0, g, lo, w, on_act=(g + li) % 2 == 0)
            k2_realign(0)
            for h in range(H3):
                v_transpose(0, h)

            # ---- phase 3: attention, software-pipelined over 18 units
            # (qc-major order: all heads of a query chunk complete together,
            # so proj chunks for those columns unlock as early as possible)
            units = [
                (e, h, qlo, qw)
                for e in range(2)
                for (qlo, qw) in QCHUNKS
                for h in range(H3)
            ]
            state = {}   # unit index -> (O tile, regions, (e,h,qlo,qw))
            pending_mult = []

            def emit_scores(i):
                e, h, qlo, qw = units[i]
                sq = 0 if e == 0 else 1
                regions = _pack_regions(_unit_plan(e, qlo, qw))
                ets = []
                for parts, used in regions:
                    rt = work.tile([128, REG_W], F32, tag="work")
                    et = expp.tile([128, REG_W], BF16, tag="expT")
                    for kind, idx, qo, loc, w in parts:
                        kv = kT(sq, h) if kind == "s" else kT(0, h)
                        p0 = 0
                        while p0 < w:
                            # 256-wide pieces: measured full-rate, while
                            # 512-wide pieces pay ~+110ns each
                            bw = min(w - p0, 256 - ((loc + p0) % 256))
                            nc.tensor.matmul(
                                rt[:, loc + p0 : loc + p0 + bw],
                                kv[:, 128 * idx : 128 * (idx + 1)],
                                qT(sq, h)[:, qo + p0 : qo + p0 + bw],
                                start=True,
                                stop=True,
                            )
                            p0 += bw
                    nc.scalar.activation(
                        et[:, 0:used],
                        rt[:, 0:used],
                        mybir.ActivationFunctionType.Exp,
                        bias=0.0,
                        scale=float(SCALE),
                    )
                    # causal corner mask fixes
                    for kind, idx, qo, loc, w in parts:
                        if kind == "s":
                            msk, mw, base = ms_sb, 128, 128 * idx
                        elif e == 0:
                            msk, mw, base = m0_sb, 128, 128 * idx
                        else:
                            msk, mw, base = m1_sb, 64, 128 * idx + 64
                        moff = qo - base
                        if moff < mw:
                            cw = min(mw - moff, w)
                            nc.vector.tensor_tensor(
                                et[:, loc : loc + cw],
                                et[:, loc : loc + cw],
                                msk[:, moff : moff + cw],
                                mybir.AluOpType.mult,
                            )
                    ets.append(et)
                state[i] = (regions, ets)

            def emit_av_norm(i):
                e, h, qlo, qw = units[i]
                sq = 0 if e == 0 else 1
                regions, ets = state.pop(i)
                O = psO.tile([128, 512], F32, tag="O")
                if e == 1 and qlo == 0:
                    nc.tensor.matmul(
                        O[0:65, 0:qw],
                        zv_sb[:],
                        on_sb[:, 0:qw],
                        start=True,
                        stop=False,
                        skip_group_check=True,
                    )
                for (parts, used), et in zip(regions, ets):
                    for kind, idx, qo, loc, w in parts:
                        vsrc = VNA[sq] if kind == "s" else VNA[0]
                        first = (
                            kind == "t"
                            and idx == 0
                            and not (e == 1 and qlo == 0)
                        )
                        if first:
                            # single matmul: exactly one start=True opens
                            # the bank's accumulation group
                            nc.tensor.matmul(
                                O[0:65, qo - qlo : qo - qlo + w],
                                vsrc[:, idx, h, :],
                                et[:, loc : loc + w],
                                start=True,
                                stop=False,
                                skip_group_check=True,
                            )
                            continue
                        p0 = 0
                        while p0 < w:
                            bw = min(w - p0, 256 - ((qo - qlo + p0) % 256))
                            nc.tensor.matmul(
                                O[0:65, qo - qlo + p0 : qo - qlo + p0 + bw],
                                vsrc[:, idx, h, :],
                                et[:, loc + p0 : loc + p0 + bw],
                                start=False,
                                stop=False,
                                skip_group_check=True,
                            )
                            p0 += bw
                # normalize: denom -> sbuf, fast recip, DRAM broadcast;
                # the final mult is deferred two units to hide DMA latency
                dn = dnp.tile([1, 512], F32, tag="dn")
                nc.vector.tensor_copy(dn[:, 0:qw], O[64:65, 0:qw])
                rrec = rrecp.tile([1, 512], F32, tag="rrec")
                nc.vector.reciprocal_approx_fast(rrec[:, 0:qw], dn[:, 0:qw])
                scr = scrp.tile([1, 512], F32, tag="scr")
                nc.sync.dma_start(scr[:, 0:qw], rrec[:, 0:qw])
                rb = rbp.tile([64, 512], F32, tag="rb")
                nc.sync.dma_start(
                    rb[:, 0:qw], scr[:, 0:qw].to_broadcast([64, qw])
                )
                target = (aT01[e][0:64, :], aT01[e][64:128, :], aT2[e][:, :])[h]
                pending_mult.append((target, qlo, qw, O, rb))

            def flush_mult():
                target, qlo, qw, O, rb = pending_mult.pop(0)
                nc.vector.tensor_tensor(
                    target[:, qlo : qlo + qw],
                    O[0:64, 0:qw],
                    rb[:, 0:qw],
                    mybir.AluOpType.mult,
                )

            def proj_chunk(e, m):
                ot = outst.tile([128, D], F32, tag="ot")
                pp = work.tile([128, REG_W], F32, tag="work")
                # all aT01 pieces first, then aT2: consecutive matmuls
                # never accumulate into the same psum columns, so each
                # drain overlaps the next fill
                for lo, w in ((0, 512), (512, 256)):
                    nc.tensor.matmul(
                        pp[:, lo : lo + w],
                        aT01[e][:, 128 * m : 128 * (m + 1)],
                        wp_sb[:, 0, lo : lo + w],
                        start=True,
                        stop=False,
                    )
                for lo, w in ((0, 512), (512, 256)):
                    nc.tensor.matmul(
                        pp[:, lo : lo + w],
                        aT2[e][:, 128 * m : 128 * (m + 1)],
                        wp_sb[0:64, 1, lo : lo + w],
                        start=False,
                        stop=True,
                    )
                nc.scalar.activation(
                    ot[:], pp[:, 0:D], COPY, bias=0.0, scale=1.0
                )
                nc.sync.dma_start(outs[e][128 * m : 128 * (m + 1), :], ot[:])

            # PE filler queues: source-1 qkv during branch-0 attention,
            # branch-0 proj during branch-1 attention (gated so each proj
            # chunk is emitted only after the normalize mults it reads)
            fq = []
            fq.append(lambda: qkv_region(1, 0, 0, 1024, False))
            fq.append(lambda: qkv_region(1, 0, 1024, 256, False))
            fq.append(lambda: k2_realign(1))
            for g in range(1, NG):
                fq.append(lambda g=g: qkv_region(1, g, 0, 1024, False))
                fq.append(lambda g=g: qkv_region(1, g, 1024, 256, False))
                if g <= 3:
                    fq.append(lambda h=g - 1: v_transpose(1, h))
            # proj chunks for query-chunk qc of branch e unlock once that
            # qc's three normalize mults have been flushed:
            # e0 qc0 -> block 6, qc1 -> block 9, qc2 -> block 12;
            # e1 qc0 -> block 15, qc1 -> block 18 (epilogue)
            fp = [lambda m=m: proj_chunk(0, m) for m in range(NP)]
            fp += [lambda m=m: proj_chunk(1, m) for m in range(NP)]
            quota_q = {i: 2 for i in range(7)}
            quota_p = {6: 2, 7: 2, 9: 2, 10: 2, 12: 2, 15: 2, 16: 2, 17: 2}

            for i in range(len(units)):
                if len(pending_mult) >= 2:
                    flush_mult()
                emit_scores(i)
                for _ in range(quota_q.get(i, 0)):
                    if fq:
                        fq.pop(0)()
                for _ in range(quota_p.get(i, 0)):
                    if fp:
                        fp.pop(0)()
                if i >= 1:
                    emit_av_norm(i - 1)
            emit_av_norm(len(units) - 1)
            while pending_mult:
                flush_mult()
            while fq:
                fq.pop(0)()
            # ---- phase 4: remaining projections
            while fp:
                fp.pop(0)()

    nc.finalize()
    return nc


# ---------------------------------------------------------------- host side

_NC = None


def _get_nc():
    global _NC
    if _NC is None:
        _NC = build()
    return _NC


def _consts():
    cbm = np.zeros((128, 991), np.float32)
    cbm[0:64, 0:64] = np.eye(64)
    cbm[64:128, 0:64] = np.eye(64)
    p = np.arange(128)[:, None]
    x = np.arange(128)[None, :]
    cbm[:, 64:192] = p <= x                       # m0
    cbm[0:64, 192:256] = 1.0                      # m1
    cbm[:, 256:384] = np.where(
        x < 64, p <= x, (p >= 64) & (p <= x)
    )                                             # ms
    cbm[:, 384:414] = 1.0                         # vo ones
    cbm[0, 414:926] = 1.0                         # on
    return dict(cb=cbm.astype(NBF))


def _core_inputs(x0, x1, w_attn, b_attn, w_proj, consts):
    """Build the 8 per-core input maps. Core order: (b, G) row-major."""
    maps = []
    xT = [
        [
            np.ascontiguousarray(x[b].reshape(T, D).T)
            .astype(NBF)
            .reshape(DC, 128, T)
            for b in range(B)
        ]
        for x in (x0, x1)
    ]
    for b in range(B):
        for G in range(4):
            gh = [3 * G + h for h in range(H3)]
            qc = [768 + g * 64 + np.arange(64) for g in gh]
            kc = [1536 + g * 64 + np.arange(64) for g in gh]
            vc = [0 + g * 64 + np.arange(64) for g in gh]
            groups = [
                np.concatenate([qc[0], kc[2]]),
                np.concatenate([qc[1], vc[0]]),
                np.concatenate([qc[2], vc[1]]),
                np.concatenate([kc[0], vc[2]]),
                np.concatenate([kc[1], kc[1]]),  # pad half unused
            ]
            cols = np.concatenate(groups)
            wqm = w_attn[:, cols].copy()
            wqm[:, 4 * 128 + 64 :] = 0.0
            bqm = b_attn[cols].reshape(NG, 128).T.copy()
            bqm[64:, 4] = 0.0
            wpm = np.zeros((2, 128, D), np.float32)
            wpm[0] = w_proj[3 * G * 64 : 3 * G * 64 + 128]
            wpm[1, 0:64] = w_proj[3 * G * 64 + 128 : 3 * G * 64 + 192]
            maps.append(
                dict(
                    xt=xT[0][b],
                    xs=xT[1][b],
                    wq=wqm.astype(NBF).reshape(DC, 128, NG * 128),
                    bq=np.ascontiguousarray(bqm, np.float32),
                    wp=wpm.astype(NBF),
                    **consts,
                )
            )
    return maps


def kernel(x0, x1, w_attn, b_attn, w_proj, b_proj, _trace=False):
    x0 = np.asarray(x0, np.float32)
    x1 = np.asarray(x1, np.float32)
    w_attn = np.asarray(w_attn, np.float32)
    b_attn = np.asarray(b_attn, np.float32)
    w_proj = np.asarray(w_proj, np.float32)
    b_proj = np.asarray(b_proj, np.float32)

    nc = _get_nc()
    maps = _core_inputs(x0, x1, w_attn, b_attn, w_proj, _consts())
    res = run_bass_kernel_spmd(nc, maps, core_ids=list(range(8)), trace=_trace)

    out = [np.zeros((B, T, D), np.float32) for _ in range(2)]
    for ci, r in enumerate(res.results):
        b = ci // 4
        out[0][b] += r["o0"]
        out[1][b] += r["o1"]
    out0 = (out[0] + b_proj).reshape(B, NB, BS, D)
    out1 = (out[1] + b_proj).reshape(B, NB, BS, D)
    if _trace:
        kernel._last = res
    return out0, out1


if __name__ == "__main__":
    rng = np.random.default_rng(0)
    x0 = rng.standard_normal((B, NB, BS, D), dtype=np.float32)
    x1 = rng.standard_normal((B, NB, BS, D), dtype=np.float32)
    wa = rng.standard_normal((D, 3 * D), dtype=np.float32) * 0.02
    ba = np.zeros(3 * D, np.float32)
    wpj = rng.standard_normal((D, D), dtype=np.float32) * 0.02
    bp_ = np.zeros(D, np.float32)
    o0, o1 = kernel(x0, x1, wa, ba, wpj, bp_)
    print("ran", o0.shape, o1.shape, float(np.abs(o0).mean()))
